# revision 1
# baseline (speedup 1.0000x reference)
"""KroneckerMessage GNN message passing on 8 TRN2 NeuronCores.

Strategy (self-contained, hardcoded for the nn_KroneckerMessage problem):
- Node phase: shard nodes 8-way; each core computes its slice of
  h = relu(LN(node_feat @ W_node + b_node)) via PE transposes + fp32 matmuls,
  then AllGather so every core has the full h table [NPAD, 20] in DRAM.
- Edge phase: shard edges by dst range (no output collective needed). Host
  buckets each core's edges into 128-node dst windows, padding every window
  to K subtiles of 128 edge slots (identical static schedule on all cores;
  per-core data differs only in input tensors).
  Per 128-edge subtile: indirect-gather h[src], h[dst] -> [128,20] fp32;
  build kron_T [128e, 400] bf16 with stride-0 broadcast APs on DVE;
  PE-transpose to [100,4x128e] bf16; 4 accumulating bf16 matmuls vs
  W_kron chunks [100,129] (col 128 = row-sums/OUT => mean for free);
  LN via ACT Square-accum (E[z^2]) + matmul mean column; normalize+relu
  fused in one ACT op -> y [128e,128o] bf16; scatter-add into the window
  via one-hot matmul accumulated in PSUM; flush each window once to DRAM.
"""
import math
import os

import numpy as np

import concourse.bacc as bacc
import concourse.bass as bass
import concourse.tile as tile
from concourse import mybir
from concourse.bass_utils import run_bass_kernel_spmd
from concourse.masks import make_identity

N_CORES = 8
P = 128
D_NODE = 20  # node projection dim
KRON = 400  # 20*20
KCH = 100  # kron rows per matmul chunk
NCH = 4  # kron chunks
LN_EPS = 1e-5

# module-level knobs (test.py pokes these)
TRACE = False
TRACE_DIR = None
USE_SIM = False

_BUILD_CACHE = {}


# --------------------------------------------------------------------------
# host-side prep
# --------------------------------------------------------------------------
def _prep(node_feat, W_node, b_node, g_node, beta_node, W_kron, b_kron,
          g_kron, beta_kron, src, dst):
    N, GF = node_feat.shape
    OUT = W_kron.shape[1]
    E = src.shape[0]
    assert GF % P == 0 and OUT == P and W_kron.shape[0] == KRON

    nodes_pc = int(math.ceil(N / (N_CORES * P))) * P
    npad = nodes_pc * N_CORES
    wpc = nodes_pc // P

    src = np.asarray(src, np.int64)
    dst = np.asarray(dst, np.int64)
    win = (dst // P).astype(np.int64)  # global window id
    counts = np.bincount(win, minlength=wpc * N_CORES)
    K = max(1, int(math.ceil(counts.max() / P)))
    slots_pw = K * P
    e_pc = wpc * slots_pw

    order = np.argsort(win, kind="stable")
    swin = win[order]
    starts = np.concatenate([[0], np.cumsum(counts)])
    rank = np.arange(E, dtype=np.int64) - starts[swin]
    slot = (swin // wpc) * e_pc + (swin % wpc) * slots_pw + rank

    src_idx = np.zeros(N_CORES * e_pc, np.int32)
    dst_idx = np.zeros(N_CORES * e_pc, np.int32)
    dst_loc = np.full(N_CORES * e_pc, -1.0, np.float32)
    src_idx[slot] = src[order].astype(np.int32)
    dst_idx[slot] = dst[order].astype(np.int32)
    dst_loc[slot] = (dst[order] % P).astype(np.float32)

    nf_pad = np.zeros((npad, GF), np.float32)
    nf_pad[:N] = np.asarray(node_feat, np.float32)

    # W_node -> [P, GF//P, D_NODE]
    wn = np.asarray(W_node, np.float32).reshape(GF // P, P, D_NODE)
    wn = np.ascontiguousarray(wn.transpose(1, 0, 2))

    # W_kron chunks with mean column: [KCH, NCH, OUT+1] bf16
    wk = np.asarray(W_kron, np.float32).reshape(NCH, KCH, OUT)
    wbar = wk.sum(axis=2, keepdims=True) / OUT
    wke = np.concatenate([wk, wbar], axis=2)  # [NCH, KCH, OUT+1]
    wke = np.ascontiguousarray(wke.transpose(1, 0, 2))

    flags = dict(
        has_bn=bool(np.any(np.asarray(b_node) != 0)),
        has_gn=bool(np.any(np.asarray(g_node) != 1)),
        has_betan=bool(np.any(np.asarray(beta_node) != 0)),
        has_bk=bool(np.any(np.asarray(b_kron) != 0)),
        has_gk=bool(np.any(np.asarray(g_kron) != 1)),
        has_betak=bool(np.any(np.asarray(beta_kron) != 0)),
    )
    bk_ext = np.concatenate(
        [np.asarray(b_kron, np.float32),
         [np.asarray(b_kron, np.float32).sum() / OUT]])

    cfg = dict(N=N, GF=GF, OUT=OUT, E=E, nodes_pc=nodes_pc, npad=npad,
               wpc=wpc, K=K, e_pc=e_pc, **flags)

    bf16 = np.dtype("bfloat16") if hasattr(np, "bfloat16") else None
    import ml_dtypes
    wke_bf = wke.astype(ml_dtypes.bfloat16)

    in_maps = []
    for c in range(N_CORES):
        m = dict(
            nf=nf_pad[c * nodes_pc:(c + 1) * nodes_pc],
            wn=wn,
            wk=wke_bf,
            src_idx=src_idx[c * e_pc:(c + 1) * e_pc],
            dst_idx=dst_idx[c * e_pc:(c + 1) * e_pc],
            dst_loc=dst_loc[c * e_pc:(c + 1) * e_pc],
            b_node=np.asarray(b_node, np.float32),
            g_node=np.asarray(g_node, np.float32),
            beta_node=np.asarray(beta_node, np.float32),
            bk=bk_ext.astype(ml_dtypes.bfloat16),
            g_kron=np.asarray(g_kron, np.float32).astype(ml_dtypes.bfloat16),
            beta_kron=np.asarray(beta_kron, np.float32).astype(ml_dtypes.bfloat16),
            iota_row=np.arange(P, dtype=np.float32),
        )
        in_maps.append(m)
    return cfg, in_maps


# --------------------------------------------------------------------------
# device program
# --------------------------------------------------------------------------
def _build(cfg):
    GF, OUT = cfg["GF"], cfg["OUT"]
    nodes_pc, npad, wpc, K, e_pc = (cfg["nodes_pc"], cfg["npad"], cfg["wpc"],
                                    cfg["K"], cfg["e_pc"])
    FCH = GF // P
    f32, bf16, i32 = mybir.dt.float32, mybir.dt.bfloat16, mybir.dt.int32
    OUTX = OUT + 1

    nc = bacc.Bacc(num_devices=N_CORES)
    nf = nc.dram_tensor("nf", [nodes_pc, GF], f32, kind="ExternalInput")
    wn = nc.dram_tensor("wn", [P, FCH, D_NODE], f32, kind="ExternalInput")
    wk = nc.dram_tensor("wk", [KCH, NCH, OUTX], bf16, kind="ExternalInput")
    src_idx = nc.dram_tensor("src_idx", [e_pc], i32, kind="ExternalInput")
    dst_idx = nc.dram_tensor("dst_idx", [e_pc], i32, kind="ExternalInput")
    dst_loc = nc.dram_tensor("dst_loc", [e_pc], f32, kind="ExternalInput")
    b_node = nc.dram_tensor("b_node", [D_NODE], f32, kind="ExternalInput")
    g_node = nc.dram_tensor("g_node", [D_NODE], f32, kind="ExternalInput")
    beta_node = nc.dram_tensor("beta_node", [D_NODE], f32, kind="ExternalInput")
    bk = nc.dram_tensor("bk", [OUTX], bf16, kind="ExternalInput")
    g_kron = nc.dram_tensor("g_kron", [OUT], bf16, kind="ExternalInput")
    iota_row = nc.dram_tensor("iota_row", [P], f32, kind="ExternalInput")
    beta_kron = nc.dram_tensor("beta_kron", [OUT], bf16, kind="ExternalInput")

    out_part = nc.dram_tensor("out_part", [nodes_pc, OUT], f32,
                              kind="ExternalOutput")
    h_part = nc.dram_tensor("h_part", [nodes_pc, D_NODE], f32)
    h_full = nc.dram_tensor("h_full", [npad, D_NODE], f32,
                            addr_space="Shared")

    ntiles = nodes_pc // P

    # ---------------- phase 1: h = relu(LN(nf @ W_node + b)) --------------
    with tile.TileContext(nc) as tc:
        with (
            tc.tile_pool(name="hconst", bufs=1) as hconst,
            tc.tile_pool(name="hsb", bufs=3) as hsb,
            tc.tile_pool(name="hps", bufs=2, space="PSUM") as hps,
            tc.tile_pool(name="hsmall", bufs=4) as hsmall,
        ):
            ident_f32 = hconst.tile([P, P], f32)
            make_identity(nc, ident_f32[:])
            wn_sb = hconst.tile([P, FCH, D_NODE], f32)
            nc.gpsimd.dma_start(out=wn_sb[:], in_=wn[:])
            eps_t = hconst.tile([P, 1], f32)
            nc.vector.memset(eps_t[:], LN_EPS)
            if cfg["has_bn"]:
                bn_b = hconst.tile([P, D_NODE], f32)
                nc.gpsimd.dma_start(
                    out=bn_b[:],
                    in_=bass.AP(tensor=b_node, offset=0,
                                ap=[[0, P], [1, D_NODE]]))
            if cfg["has_gn"]:
                gn_b = hconst.tile([P, D_NODE], f32)
                nc.gpsimd.dma_start(
                    out=gn_b[:],
                    in_=bass.AP(tensor=g_node, offset=0,
                                ap=[[0, P], [1, D_NODE]]))
            if cfg["has_betan"]:
                betan_b = hconst.tile([P, D_NODE], f32)
                nc.gpsimd.dma_start(
                    out=betan_b[:],
                    in_=bass.AP(tensor=beta_node, offset=0,
                                ap=[[0, P], [1, D_NODE]]))

            h_stage = hconst.tile([P, ntiles, D_NODE], f32)

            for t in range(ntiles):
                nf_t = hsb.tile([P, GF], f32, tag="nf_t")
                nc.gpsimd.dma_start(out=nf_t[:], in_=nf[t * P:(t + 1) * P, :])
                nfT_ps = hps.tile([P, FCH, P], f32, tag="nfT_ps")
                for c in range(FCH):
                    nc.tensor.transpose(out=nfT_ps[:, c, :],
                                        in_=nf_t[:, c * P:(c + 1) * P],
                                        identity=ident_f32[:])
                nfT = hsb.tile([P, FCH, P], f32, tag="nfT")
                nc.vector.tensor_copy(out=nfT[:], in_=nfT_ps[:])
                z_ps = hps.tile([P, D_NODE], f32, tag="z_ps")
                for c in range(FCH):
                    nc.tensor.matmul(out=z_ps[:], lhsT=nfT[:, c, :],
                                     rhs=wn_sb[:, c, :], start=(c == 0),
                                     stop=(c == FCH - 1))
                if cfg["has_bn"]:
                    z_sb = hsb.tile([P, D_NODE], f32, tag="z_sb")
                    nc.vector.tensor_add(out=z_sb[:], in0=z_ps[:], in1=bn_b[:])
                    z_in = z_sb
                else:
                    z_in = z_ps
                stats = hsmall.tile([P, 6], f32, tag="stats")
                nc.vector.bn_stats(out=stats[:], in_=z_in[:])
                mv = hsmall.tile([P, 2], f32, tag="mv")
                nc.vector.bn_aggr(out=mv[:], in_=stats[:])
                sd = hsmall.tile([P, 1], f32, tag="sd")
                nc.scalar.activation(out=sd[:], in_=mv[:, 1:2],
                                     func=mybir.ActivationFunctionType.Sqrt,
                                     bias=eps_t[:], scale=1.0)
                rstd = hsmall.tile([P, 1], f32, tag="rstd")
                nc.vector.reciprocal(out=rstd[:], in_=sd[:])
                nmr = hsmall.tile([P, 1], f32, tag="nmr")
                nc.vector.tensor_scalar(out=nmr[:], in0=mv[:, 0:1],
                                        scalar1=rstd[:, 0:1], scalar2=-1.0,
                                        op0=mybir.AluOpType.mult,
                                        op1=mybir.AluOpType.mult)
                simple = not (cfg["has_gn"] or cfg["has_betan"])
                func = (mybir.ActivationFunctionType.Relu if simple
                        else mybir.ActivationFunctionType.Identity)
                nc.scalar.activation(out=h_stage[:, t, :], in_=z_in[:],
                                     func=func, bias=nmr[:],
                                     scale=rstd[:, 0:1])
                if not simple:
                    if cfg["has_gn"]:
                        nc.vector.tensor_mul(out=h_stage[:, t, :],
                                             in0=h_stage[:, t, :],
                                             in1=gn_b[:])
                    if cfg["has_betan"]:
                        nc.vector.tensor_add(out=h_stage[:, t, :],
                                             in0=h_stage[:, t, :],
                                             in1=betan_b[:])
                    nc.vector.tensor_scalar_max(out=h_stage[:, t, :],
                                                in0=h_stage[:, t, :],
                                                scalar1=0.0)
            nc.sync.dma_start(
                out=h_part.rearrange("(t p) d -> p t d", p=P),
                in_=h_stage[:])

    # ---------------- collective: AllGather h ----------------------------
    with (
        nc.Block() as block,
        nc.semaphore("cc_sem") as cc_sem,
    ):
        @block.gpsimd
        def _(gpsimd):
            gpsimd.collective_compute(
                "AllGather",
                mybir.AluOpType.bypass,
                replica_groups=[list(range(N_CORES))],
                ins=[h_part[:]],
                outs=[h_full[:]],
            ).then_inc(cc_sem)
            gpsimd.wait_ge(cc_sem, 1)

    # ---------------- phase 2: edges --------------------------------------
    simple_k = not (cfg["has_gk"] or cfg["has_betak"])
    with tile.TileContext(nc) as tc:
        with (
            tc.tile_pool(name="econst", bufs=1) as econst,
            tc.tile_pool(name="eg", bufs=2) as eg,
            tc.tile_pool(name="esb", bufs=3) as esb,
            tc.tile_pool(name="eps_t", bufs=2, space="PSUM") as epsT,
            tc.tile_pool(name="eps_z", bufs=2, space="PSUM") as epsZ,
            tc.tile_pool(name="eps_a", bufs=2, space="PSUM") as epsA,
            tc.tile_pool(name="esmall", bufs=6) as esmall,
        ):
            ident_bf = econst.tile([P, P], bf16)
            make_identity(nc, ident_bf[:])
            iota_f = econst.tile([P, P], f32)
            nc.gpsimd.dma_start(
                out=iota_f[:], in_=bass.AP(tensor=iota_row, offset=0,
                                           ap=[[0, P], [1, P]]))
            eps_t2 = econst.tile([P, 1], f32)
            nc.vector.memset(eps_t2[:], LN_EPS)
            wk_sb = econst.tile([KCH, NCH, OUTX], bf16)
            nc.gpsimd.dma_start(out=wk_sb[:], in_=wk[:])
            if cfg["has_bk"]:
                ones_row = econst.tile([1, P], bf16)
                nc.vector.memset(ones_row[:], 1.0)
                bk_sb = econst.tile([1, OUTX], bf16)
                nc.gpsimd.dma_start(out=bk_sb[:], in_=bk[None, :])
            if cfg["has_gk"]:
                gk_b = econst.tile([P, OUT], bf16)
                nc.gpsimd.dma_start(
                    out=gk_b[:], in_=bass.AP(tensor=g_kron, offset=0,
                                             ap=[[0, P], [1, OUT]]))
            if cfg["has_betak"]:
                betak_b = econst.tile([P, OUT], bf16)
                nc.gpsimd.dma_start(
                    out=betak_b[:],
                    in_=bass.AP(tensor=beta_kron, offset=0,
                                ap=[[0, P], [1, OUT]]))

            for w in range(wpc):
                base = w * K * P
                sidx = eg.tile([P, K], i32, tag="sidx")
                nc.sync.dma_start(
                    out=sidx[:],
                    in_=bass.AP(tensor=src_idx, offset=base,
                                ap=[[1, P], [P, K]]))
                didx = eg.tile([P, K], i32, tag="didx")
                nc.sync.dma_start(
                    out=didx[:],
                    in_=bass.AP(tensor=dst_idx, offset=base,
                                ap=[[1, P], [P, K]]))
                dloc = eg.tile([P, K], f32, tag="dloc")
                nc.sync.dma_start(
                    out=dloc[:],
                    in_=bass.AP(tensor=dst_loc, offset=base,
                                ap=[[1, P], [P, K]]))
                hs = eg.tile([P, K, D_NODE], f32, tag="hs")
                hd = eg.tile([P, K, D_NODE], f32, tag="hd")
                for s in range(K):
                    # HW indirect DMA honors one index per partition only.
                    nc.gpsimd.indirect_dma_start(
                        out=hs[:, s, :], out_offset=None, in_=h_full[:],
                        in_offset=bass.IndirectOffsetOnAxis(
                            ap=sidx[:, s:s + 1], axis=0))
                    nc.gpsimd.indirect_dma_start(
                        out=hd[:, s, :], out_offset=None, in_=h_full[:],
                        in_offset=bass.IndirectOffsetOnAxis(
                            ap=didx[:, s:s + 1], axis=0))

                acc_ps = epsA.tile([P, OUT], f32, tag="acc")

                for s in range(K):
                    kronT = esb.tile([P, KRON], bf16, tag="kronT")
                    kv = kronT[:].rearrange("p (a b) -> p a b", a=D_NODE)
                    nc.vector.tensor_tensor(
                        out=kv,
                        in0=hs[:, s, :, None].to_broadcast(
                            [P, D_NODE, D_NODE]),
                        in1=hd[:, s, None, :].to_broadcast(
                            [P, D_NODE, D_NODE]),
                        op=mybir.AluOpType.mult)
                    psT = epsT.tile([KCH, NCH, P], bf16, tag="psT")
                    for c in range(NCH):
                        nc.tensor.transpose(
                            out=psT[:, c, :],
                            in_=kronT[:, c * KCH:(c + 1) * KCH],
                            identity=ident_bf[:])
                    kron_sb = esb.tile([KCH, NCH, P], bf16, tag="kron_sb")
                    nc.vector.tensor_copy(out=kron_sb[:], in_=psT[:])

                    z_ps = epsZ.tile([P, OUTX], f32, tag="z")
                    nmm = NCH + (1 if cfg["has_bk"] else 0)
                    for c in range(NCH):
                        nc.tensor.matmul(out=z_ps[:], lhsT=kron_sb[:, c, :],
                                         rhs=wk_sb[:, c, :], start=(c == 0),
                                         stop=(c == nmm - 1))
                    if cfg["has_bk"]:
                        nc.tensor.matmul(out=z_ps[:], lhsT=ones_row[:],
                                         rhs=bk_sb[:], start=False, stop=True,
                                         skip_group_check=True)

                    # LN stats: mean = z_ps[:, OUT]; E[z^2] via ACT square
                    sumsq = esmall.tile([P, 1], f32, tag="sumsq")
                    sq_trash = esb.tile([P, OUT], bf16, tag="sq_trash")
                    nc.scalar.activation(
                        out=sq_trash[:], in_=z_ps[:, 0:OUT],
                        func=mybir.ActivationFunctionType.Square,
                        scale=float(1.0 / math.sqrt(OUT)),
                        accum_out=sumsq[:])
                    mu_sb = esmall.tile([P, 1], f32, tag="mu_sb")
                    nc.vector.tensor_copy(out=mu_sb[:], in_=z_ps[:, OUT:OUTX])
                    musq = esmall.tile([P, 1], f32, tag="musq")
                    nc.vector.tensor_tensor(out=musq[:], in0=mu_sb[:],
                                            in1=mu_sb[:],
                                            op=mybir.AluOpType.mult)
                    var = esmall.tile([P, 1], f32, tag="var")
                    nc.vector.tensor_tensor(out=var[:], in0=sumsq[:],
                                            in1=musq[:],
                                            op=mybir.AluOpType.subtract)
                    sd = esmall.tile([P, 1], f32, tag="sd2")
                    nc.scalar.activation(
                        out=sd[:], in_=var[:],
                        func=mybir.ActivationFunctionType.Sqrt,
                        bias=eps_t2[:], scale=1.0)
                    rstd = esmall.tile([P, 1], f32, tag="rstd2")
                    nc.vector.reciprocal(out=rstd[:], in_=sd[:])
                    nmr = esmall.tile([P, 1], f32, tag="nmr2")
                    nc.vector.tensor_scalar(out=nmr[:], in0=mu_sb[:],
                                            scalar1=rstd[:, 0:1],
                                            scalar2=-1.0,
                                            op0=mybir.AluOpType.mult,
                                            op1=mybir.AluOpType.mult)
                    y_sb = esb.tile([P, OUT], bf16, tag="y")
                    func = (mybir.ActivationFunctionType.Relu if simple_k
                            else mybir.ActivationFunctionType.Identity)
                    nc.scalar.activation(out=y_sb[:], in_=z_ps[:, 0:OUT],
                                         func=func, bias=nmr[:],
                                         scale=rstd[:, 0:1])
                    if not simple_k:
                        if cfg["has_gk"]:
                            nc.vector.tensor_mul(out=y_sb[:], in0=y_sb[:],
                                                 in1=gk_b[:])
                        if cfg["has_betak"]:
                            nc.vector.tensor_add(out=y_sb[:], in0=y_sb[:],
                                                 in1=betak_b[:])
                        nc.vector.tensor_scalar_max(out=y_sb[:], in0=y_sb[:],
                                                    scalar1=0.0)

                    oh = esb.tile([P, P], bf16, tag="oh")
                    nc.vector.tensor_scalar(out=oh[:], in0=iota_f[:],
                                            scalar1=dloc[:, s:s + 1],
                                            scalar2=None,
                                            op0=mybir.AluOpType.is_equal)
                    nc.tensor.matmul(out=acc_ps[:], lhsT=oh[:], rhs=y_sb[:],
                                     start=(s == 0), stop=(s == K - 1))

                out_sb = esb.tile([P, OUT], f32, tag="out_sb")
                nc.vector.tensor_copy(out=out_sb[:], in_=acc_ps[:])
                nc.sync.dma_start(out=out_part[w * P:(w + 1) * P, :],
                                  in_=out_sb[:])

    nc.compile()
    return nc


# --------------------------------------------------------------------------
# entry point
# --------------------------------------------------------------------------
def _install_trace_hook():
    import sys, types, ctypes, contextlib
    if "antenv.axon_hooks" in sys.modules:
        return
    lib = ctypes.CDLL("/opt/axon/libaxon_pjrt.so")
    lib.axon_start_nrt_profile.argtypes = [ctypes.POINTER(ctypes.c_int64),
                                           ctypes.c_size_t]
    lib.axon_start_nrt_profile.restype = ctypes.c_int64
    lib.axon_stop_nrt_profile.argtypes = [ctypes.c_char_p]
    lib.axon_stop_nrt_profile.restype = ctypes.c_int64

    @contextlib.contextmanager
    def _hook(output_dir, device_ids):
        import jax
        jax.devices()
        if device_ids:
            ids = (ctypes.c_int64 * len(device_ids))(*device_ids)
            rc = lib.axon_start_nrt_profile(ids, len(device_ids))
        else:
            rc = lib.axon_start_nrt_profile(None, 0)
        if rc != 0:
            raise RuntimeError(f"axon_start_nrt_profile rc={rc}")
        try:
            yield
        finally:
            n = lib.axon_stop_nrt_profile(str(output_dir).encode())
            print(f"profile: {n} file(s) -> {output_dir}")

    mod = types.ModuleType("antenv.axon_hooks")
    mod.get_axon_ntff_profile_hook = lambda: _hook
    sys.modules["antenv.axon_hooks"] = mod
    from concourse import bass_utils
    bass_utils.upload_artifacts = lambda tmpdir: "local://skipped"


def kernel(**inputs):
    cfg, in_maps = _prep(**inputs)
    key = (cfg["N"], cfg["GF"], cfg["OUT"], cfg["K"], cfg["e_pc"],
           cfg["has_bn"], cfg["has_gn"], cfg["has_betan"], cfg["has_bk"],
           cfg["has_gk"], cfg["has_betak"])
    if key not in _BUILD_CACHE:
        _BUILD_CACHE[key] = _build(cfg)
    nc = _BUILD_CACHE[key]

    if USE_SIM:
        from concourse import bass_interp
        sim = bass_interp.MultiCoreSim(nc, N_CORES)
        for c in range(N_CORES):
            for name, arr in in_maps[c].items():
                sim.cores[c].tensor(name)[:] = arr
        sim.simulate()
        parts = [np.array(sim.cores[c].tensor("out_part"))
                 for c in range(N_CORES)]
        exec_ns = None
    else:
        kw = {}
        if TRACE:
            _install_trace_hook()
            kw = dict(trace=True, tmpdir=TRACE_DIR)
        res = run_bass_kernel_spmd(nc, in_maps, list(range(N_CORES)), **kw)
        parts = [res.results[c]["out_part"] for c in range(N_CORES)]
        exec_ns = res.exec_time_ns
        kernel.last_exec_ns = exec_ns

    out = np.concatenate(parts, axis=0)[:cfg["N"]]
    return out.astype(np.float32)


kernel.last_exec_ns = None

